# revision 1
# baseline (speedup 1.0000x reference)
"""Trainium2 Bass kernel for the dense real-space long-range kernel
(N=6144 atoms, B=8 periodic cells, screened-Coulomb pair energy with
minimum-image convention, row-summed per atom).

Strategy: batch is sorted, and cross-graph pairs are masked out by the
reference, so the N x N problem is block-diagonal over the 8 graphs.
One graph per NeuronCore.  All math is done in fractional coordinates:

  f_k[i,j]  = frac_k[j] - frac_k[i]          (DVE tensor_scalar, broadcast row)
  r_k       = round(f_k)                     (DVE magic-number round)
  y         = (f - r) @ C                    (2 accumulating fp32r matmuls,
                                              block-diag cell stationary)
  q         = sum_j y_j^2                    (ACT Square + ones-blockdiag matmul)
  kern      = exp(-sigma*sqrt(q+soft^2)) / sqrt(q+soft^2)
                                             (ACT Sqrt/Exp + DVE recip approx)
  acc[j]    = sum_i src_i * kern[i,j]        (fp32r matvec, PSUM accumulate
                                              over row blocks; row sum == col
                                              sum by symmetry)
  E[j]      = 0.5*src_j*acc_j - 0.5*src_j^2*exp(-sigma*soft)/soft

Atoms are processed in groups of 32 (3 coordinate rows per atom = 96
partitions); 4 groups form a 128-atom macro block whose q tile uses the
full partition width for the transcendental tail.  The 32-atom group size
matches the PE tile_position granularity for the stage-2 row offsets.
"""
import numpy as np

GA = 42            # atoms per k-interleaved row group
ROWS = 3 * GA      # 126 partitions per group tile
GPM = 3            # groups per macro block
MACRO = GA * GPM   # 126 atoms per macro
MAGIC = 12582912.0  # 1.5 * 2**23: (x + MAGIC) - MAGIC == round(x) for |x| < 2**22
NCORES = 8
CHUNK = 512        # PSUM bank / fp32 matmul free-dim limit

_cache = {}


def _build(n_macros, cols, sigma, soft):
    import concourse.bacc as bacc
    import concourse.mybir as mybir
    import concourse.tile as tile

    f32 = mybir.dt.float32
    f32r = mybir.dt.float32r
    alu = mybir.AluOpType
    act = mybir.ActivationFunctionType

    n_groups = GPM * n_macros
    pw = -(-cols // CHUNK) * CHUNK
    chunks = [(c, min(cols, c + CHUNK)) for c in range(0, cols, CHUNK)]
    soft2 = float(np.float32(soft) * np.float32(soft))

    nc = bacc.Bacc("TRN2", target_bir_lowering=False, debug=False)
    # const AP for the Sqrt bias (soft^2), registered like the built-ins
    t = nc.alloc_sbuf_tensor("const-soft2", [128, 1], f32)
    nc.gpsimd.memset(t.ap(), soft2)
    nc.const_aps.aps[(f32, soft2)] = t.ap()
    nc.all_engine_barrier()

    FB = nc.declare_dram_parameter("FB", [ROWS, cols], f32, isOutput=False)
    NEGFA = nc.declare_dram_parameter("NEGFA", [ROWS, n_groups], f32, isOutput=False)
    CB = nc.declare_dram_parameter("CB", [ROWS, ROWS], f32r, isOutput=False)
    CBN = nc.declare_dram_parameter("CBN", [ROWS, ROWS], f32r, isOutput=False)
    ONESB = nc.declare_dram_parameter("ONESB", [ROWS, GPM * MACRO], f32r, isOutput=False)
    SRCST = nc.declare_dram_parameter("SRCST", [MACRO, n_macros], f32r, isOutput=False)
    A1 = nc.declare_dram_parameter("A1", [1, cols], f32, isOutput=False)
    A2 = nc.declare_dram_parameter("A2", [1, cols], f32, isOutput=False)
    OUT = nc.declare_dram_parameter("OUT", [1, cols], f32, isOutput=True)

    with tile.TileContext(nc) as tc:
        with tc.tile_pool(name="const", bufs=1) as cpool, \
             tc.tile_pool(name="work", bufs=3) as pool, \
             tc.tile_pool(name="ypsum", bufs=2, space="PSUM") as ypool, \
             tc.tile_pool(name="qpsum", bufs=1, space="PSUM") as qpool, \
             tc.tile_pool(name="apsum", bufs=1, space="PSUM") as apool:
            fb = cpool.tile([ROWS, cols], f32)
            negfa = cpool.tile([ROWS, n_groups], f32)
            cb = cpool.tile([ROWS, ROWS], f32r)
            cbn = cpool.tile([ROWS, ROWS], f32r)
            onesb = cpool.tile([ROWS, GPM * MACRO], f32r)
            srcst = cpool.tile([MACRO, n_macros], f32r)
            a1 = cpool.tile([1, cols], f32)
            a2 = cpool.tile([1, cols], f32)
            nc.sync.dma_start(fb[:], FB[:])
            nc.sync.dma_start(negfa[:], NEGFA[:])
            nc.sync.dma_start(cb[:], CB[:])
            nc.sync.dma_start(cbn[:], CBN[:])
            nc.sync.dma_start(onesb[:], ONESB[:])
            nc.sync.dma_start(srcst[:], SRCST[:])
            nc.sync.dma_start(a1[:], A1[:])
            nc.sync.dma_start(a2[:], A2[:])

            acc = apool.tile([1, pw], f32)

            for m in range(n_macros):
                q = qpool.tile([MACRO, pw], f32, tag="q")
                for tgi in range(GPM):
                    g = GPM * m + tgi
                    f = pool.tile([ROWS, cols], f32r, tag="f")
                    nc.vector.tensor_scalar(f[:], fb[:], negfa[:, g:g + 1], None,
                                            alu.add)
                    r = pool.tile([ROWS, cols], f32r, tag="r")
                    nc.vector.tensor_scalar(r[:], f[:], MAGIC, MAGIC,
                                            alu.add, alu.subtract)
                    y = ypool.tile([ROWS, pw], f32, tag="y")
                    for (c0, c1) in chunks:
                        nc.tensor.matmul(y[:, c0:c1], cb[:], f[:, c0:c1],
                                         start=True, stop=False)
                        nc.tensor.matmul(y[:, c0:c1], cbn[:], r[:, c0:c1],
                                         start=False, stop=True)
                    sq = pool.tile([ROWS, cols], f32r, tag="sq")
                    nc.scalar.activation(sq[:], y[:, 0:cols], act.Square)
                    # ones-blockdiag zero-padded to map group tgi's atoms to
                    # q rows 32*tgi..32*tgi+31; accumulate all 4 groups
                    ob = onesb[:, MACRO * tgi:MACRO * (tgi + 1)]
                    for (c0, c1) in chunks:
                        nc.tensor.matmul(q[:, c0:c1], ob, sq[:, c0:c1],
                                         start=(tgi == 0), stop=(tgi == GPM - 1))
                rt = pool.tile([MACRO, cols], f32, tag="rt")
                nc.scalar.activation(rt[:], q[:, 0:cols], act.Sqrt, bias=soft2)
                et = pool.tile([MACRO, cols], f32, tag="et")
                nc.scalar.activation(et[:], rt[:], act.Exp, scale=-sigma)
                rcp = pool.tile([MACRO, cols], f32, tag="rcp")
                nc.vector.reciprocal_approx_fast(out=rcp[:], in_=rt[:])
                kern = pool.tile([MACRO, cols], f32r, tag="kern")
                nc.vector.tensor_tensor(kern[:], et[:], rcp[:], alu.mult)
                for (c0, c1) in chunks:
                    nc.tensor.matmul(acc[0:1, c0:c1], srcst[:, m:m + 1],
                                     kern[:, c0:c1],
                                     start=(m == 0), stop=(m == n_macros - 1))

            t1 = pool.tile([1, cols], f32, tag="t1")
            nc.vector.tensor_tensor(t1[:], acc[0:1, 0:cols], a1[:], alu.mult)
            eo = pool.tile([1, cols], f32, tag="eo")
            nc.vector.tensor_tensor(eo[:], t1[:], a2[:], alu.subtract)
            nc.sync.dma_start(OUT[:], eo[:])
    nc.compile()
    return nc


def _get_program(n_macros, cols, sigma, soft):
    key = (n_macros, cols, round(sigma, 9), round(soft, 9))
    if key not in _cache:
        _cache[key] = _build(n_macros, cols, sigma, soft)
    return _cache[key]


LAST_EXEC_TIME_NS = None


def kernel(pos, batch, cell, source, screening, softening, *, _trace=False):
    global LAST_EXEC_TIME_NS
    from concourse.bass_utils import run_bass_kernel_spmd

    pos = np.asarray(pos)
    batch = np.asarray(batch)
    cell = np.asarray(cell)
    source = np.asarray(source, dtype=np.float32)
    sigma = float(np.asarray(screening, dtype=np.float32))
    soft = float(np.asarray(softening, dtype=np.float32))

    n = pos.shape[0]
    nb = cell.shape[0]
    bi = batch.astype(np.int64)
    counts = np.bincount(bi, minlength=nb)
    starts = np.concatenate([[0], np.cumsum(counts)])
    assert nb == NCORES and np.all(np.diff(bi) >= 0)

    # host precompute in float64
    inv = np.linalg.inv(cell.astype(np.float64))
    frac = np.empty((n, 3), dtype=np.float64)
    for g in range(nb):
        i0, i1 = starts[g], starts[g + 1]
        frac[i0:i1] = pos[i0:i1].astype(np.float64) @ inv[g]
    frac32 = frac.astype(np.float32)

    namax = int(counts.max())
    n_macros = -(-namax // MACRO)
    cols = MACRO * n_macros       # padded atom count per core
    n_groups = GPM * n_macros
    diag_c = float(np.exp(-np.float64(sigma) * np.float64(soft)) / np.float64(soft))

    idx_atom = np.arange(ROWS) // 3
    idx_k = np.arange(ROWS) % 3

    in_maps = []
    for g in range(nb):
        i0, i1 = starts[g], starts[g + 1]
        ng = i1 - i0
        fpad = np.zeros((cols, 3), dtype=np.float32)
        fpad[:ng] = frac32[i0:i1]
        spad = np.zeros(cols, dtype=np.float32)
        spad[:ng] = source[i0:i1]

        fb = np.ascontiguousarray(np.tile(fpad.T, (GA, 1)))  # [96, cols]: row p -> coord p%3
        negfa = np.zeros((ROWS, n_groups), dtype=np.float32)
        for t in range(n_groups):
            a = t * GA + idx_atom
            negfa[:, t] = -fpad[a, idx_k]
        C = cell[g].astype(np.float32)
        cbm = np.zeros((ROWS, ROWS), dtype=np.float32)
        for i in range(GA):
            cbm[3 * i:3 * i + 3, 3 * i:3 * i + 3] = C
        onesb = np.zeros((ROWS, GPM, MACRO), dtype=np.float32)
        for t in range(GPM):
            for i in range(GA):
                onesb[3 * i:3 * i + 3, t, GA * t + i] = 1.0
        onesb = np.ascontiguousarray(onesb.reshape(ROWS, GPM * MACRO))
        srcst = np.zeros((MACRO, n_macros), dtype=np.float32)
        for m in range(n_macros):
            srcst[:, m] = spad[m * MACRO: m * MACRO + MACRO]
        a1 = (0.5 * spad)[None, :].astype(np.float32)
        a2 = (0.5 * spad.astype(np.float64) ** 2 * diag_c)[None, :].astype(np.float32)
        in_maps.append({
            "FB": fb, "NEGFA": negfa, "CB": cbm, "CBN": -cbm,
            "ONESB": onesb, "SRCST": srcst, "A1": a1, "A2": a2,
        })

    nc = _get_program(n_macros, cols, sigma, soft)
    res = run_bass_kernel_spmd(nc, in_maps, list(range(NCORES)), trace=_trace)
    LAST_EXEC_TIME_NS = res.exec_time_ns

    out = np.zeros((n, 1), dtype=np.float32)
    for g in range(nb):
        i0, i1 = starts[g], starts[g + 1]
        out[i0:i1, 0] = res.results[g]["OUT"][0, : i1 - i0]
    return out

